# revision 5
# baseline (speedup 1.0000x reference)
"""AUROC surrogate loss on 8 TRN2 NeuronCores.

Reference (for s = sigmoid(y_pred), pos = y_true==1, neg = y_true==0):
    total = sum_{i in pos, j in neg} relu(1 - (s_i - s_j));  loss = total/(P*Q)

Because s in [0,1], s_i - s_j in [-1,1], so 1 - (s_i - s_j) >= 0 ALWAYS and the
relu never clips.  The O(N^2) pairwise sum is therefore exactly linear:
    total = P*Q - Q*S_pos + P*S_neg   =>   loss = 1 - S_pos/P + S_neg/Q
with S_pos/S_neg the sums of s over the positive/negative class.  This turns
the problem into an O(N) streaming reduction (memory-bound), which is what the
device computes.

Sharding strategy (data-parallel over the positive/negative axes, per the
problem hint): each core gets a contiguous 1/8 shard; within the shard the
host packs positive elements into whole 128-wide partitions first, then
negatives (slack filled with -40, whose sigmoid ~ 4e-18 is negligible).  The
device kernel per core is a single fused pass:
    DMA in [17,128] -> ScalarE sigmoid with accum_out (per-partition sums)
    -> DMA out [17,1]
The per-class partition split point is host bookkeeping; the host all-reduces
the 8 cores' partial sums and applies the closed-form formula (P and Q are
label counts, computed host-side as part of the unshard).

Engine program notes (hard-won on real silicon):
  * the ACT memzero is the first ScalarE op so walrus hoists the sigmoid PWP
    table load (~1.3us) to t~0 where it overlaps the input DMA
  * the output DMA is issued by ScalarE itself, in order after the
    activation+accum -- no cross-engine semaphore hop
  * no completion wait on the output DMA: the NEFF's multi-microsecond
    engine-teardown sequence runs after the module body, giving the 68-byte
    transfer ample time to land before execution completes (validated over
    many runs); this keeps ~1us of HWDGE completion latency off the
    critical path.
"""

import numpy as np

N = 16384
N_CORES = 8
SHARD = N // N_CORES  # 2048
PARTS = 17  # ceil(pos/128) + ceil(neg/128) <= 17 for any split of 2048
F = 128
PAD = np.float32(-40.0)  # sigmoid(-40) ~ 4.25e-18

_NC_CACHE = {}


def build_nc():
    import concourse.bass as bass
    from concourse import mybir

    nc = bass.Bass(num_devices=N_CORES, enable_partition_id=False)
    yp = nc.dram_tensor("yp_packed", [PARTS, F], mybir.dt.float32, kind="ExternalInput")
    out = nc.dram_tensor("psums", [PARTS, 1], mybir.dt.float32, kind="ExternalOutput")

    with (
        nc.sbuf_tensor([PARTS, F], mybir.dt.float32) as ypt,
        nc.sbuf_tensor([PARTS, F], mybir.dt.float32) as s,
        nc.sbuf_tensor([PARTS, 1], mybir.dt.float32) as red,
        nc.sbuf_tensor([PARTS, 1], mybir.dt.float32) as bias,
        nc.semaphore() as sp,
        nc.semaphore() as so,
        nc.Block() as block,
    ):

        @block.scalar
        def _(scalar):
            # first ACT op: walrus inserts the PWP table load before it,
            # overlapping the input DMA; also zeroes the sigmoid bias AP
            scalar.memzero(bias[:])
            scalar.drain()  # bias write retired before the activation reads it
            scalar.wait_ge(sp, 16)
            scalar.activation(
                out=s[:],
                in_=ypt[:],
                func=mybir.ActivationFunctionType.Sigmoid,
                bias=bias[:],
                accum_out=red[:],  # per-partition sum of sigmoid
            )
            # issued in order after the activation+accumulator-read completes
            scalar.dma_start(out=out.ap(), in_=red[:]).then_inc(so, 16)

        @block.sync
        def _(sync):
            sync.dma_start(out=ypt[:], in_=yp.ap()).then_inc(sp, 16)

    return nc


def get_nc():
    if "nc" not in _NC_CACHE:
        _NC_CACHE["nc"] = build_nc()
    return _NC_CACHE["nc"]


def _pack_shard(yp_shard, yt_shard):
    """Pack one core's shard: positives fill whole partitions first (padded),
    then negatives (padded).  Returns (packed [PARTS,F] f32, n_pos_partitions).
    """
    pos = yp_shard[yt_shard == 1]
    neg = yp_shard[yt_shard == 0]
    pos_parts = (len(pos) + F - 1) // F
    packed = np.full((PARTS, F), PAD, dtype=np.float32)
    flat = packed.reshape(-1)
    flat[: len(pos)] = pos
    flat[pos_parts * F : pos_parts * F + len(neg)] = neg
    return packed, pos_parts


def kernel(y_pred, y_true):
    from concourse import bass_utils

    y_pred = np.asarray(y_pred, dtype=np.float32).reshape(N)
    y_true = np.asarray(y_true, dtype=np.int32).reshape(N)

    in_maps = []
    pos_parts = []
    for i in range(N_CORES):
        sl = slice(i * SHARD, (i + 1) * SHARD)
        packed, pp = _pack_shard(y_pred[sl], y_true[sl])
        in_maps.append({"yp_packed": packed})
        pos_parts.append(pp)

    nc = get_nc()
    res = bass_utils.run_bass_kernel_spmd(nc, in_maps, core_ids=list(range(N_CORES)))

    s_pos = 0.0
    s_neg = 0.0
    for pp, r in zip(pos_parts, res.results):
        psums = np.asarray(r["psums"], dtype=np.float64).reshape(-1)
        s_pos += psums[:pp].sum()
        s_neg += psums[pp:].sum()

    p_cnt = float((y_true == 1).sum())
    q_cnt = float((y_true == 0).sum())
    if p_cnt * q_cnt <= 0:
        return np.array(0.0, dtype=np.float32)
    loss = 1.0 - s_pos / p_cnt + s_neg / q_cnt
    return np.array(loss, dtype=np.float32)
